# revision 2
# baseline (speedup 1.0000x reference)
"""Trainium2 Bass kernel for nn_DeformParams (gnn_message_passing).

Math (per sampled vertex s, neighbors d=0..15):
    rotated[s,d] = R[s] @ (v[s] - vn[s,d])
    new_verts[s] = sum_d w[s,d] * (rotated[s,d] + vn[s,d] + t[s])
which factors to
    u[s]  = sum_d w[s,d] * vn[s,d]          (the only gather-dependent term)
    W[s]  = sum_d w[s,d]
    new_verts[s] = W*(R@v + t) - R@u + u
    updated = verts with rows sampled_ids replaced by new_verts

Distribution: sampled vertices (S=500k) sharded across 8 NeuronCores; verts
table replicated (stays in HBM, rows fetched by indirect DMA gather).

Device kernel per core: for each tile of 128 sampled vertices x 16 neighbors,
gather the 128 neighbor rows per d via gpsimd indirect DMA ([128,1]-offset
form, one row per partition), then DVE does the weighted reduction and the
3x3 dense math. Output is the core's new_verts slice; `updated` is assembled
on the host from new_verts + untouched verts rows (output assembly only).
"""
import numpy as np

N_CORES = 8
S_FULL = 500_000
DEG = 16
NV = 2_000_000

S_SH = S_FULL // N_CORES          # 62500 sampled verts per core
TILES = (S_SH + 127) // 128       # 489 tiles of 128
S_PAD = TILES * 128               # 62592 (padded)
G = 163                           # s-tiles per super-tile; 489 = 3 * 163
N_SUPER = TILES // G              # 3

_nc_cache = {}


def _build_nc():
    import concourse.bacc as bacc
    import concourse.bass as bass
    import concourse.mybir as mybir
    import concourse.tile as tile

    nc = bacc.Bacc("TRN2", target_bir_lowering=False, debug=False,
                   num_devices=N_CORES)
    f32 = mybir.dt.float32
    verts_d = nc.dram_tensor("verts", [NV, 3], f32, kind="ExternalInput")
    # per-core, s-padded, partition-last layouts (host pre-transposed)
    nbr_d = nc.dram_tensor("nbr", [128, TILES, DEG], mybir.dt.int32, kind="ExternalInput")
    w_d = nc.dram_tensor("w", [128, TILES, DEG], f32, kind="ExternalInput")
    r_d = nc.dram_tensor("r", [128, TILES, 9], f32, kind="ExternalInput")
    t_d = nc.dram_tensor("t", [128, TILES, 3], f32, kind="ExternalInput")
    v_d = nc.dram_tensor("v", [128, TILES, 3], f32, kind="ExternalInput")
    out_d = nc.dram_tensor("out", [128, TILES, 3], f32, kind="ExternalOutput")

    with tile.TileContext(nc) as tc:
        with tc.tile_pool(name="io", bufs=2) as io_p, \
             tc.tile_pool(name="gat", bufs=2) as gat_p, \
             tc.tile_pool(name="scr", bufs=2) as scr_p:
            for sup in range(N_SUPER):
                t0 = sup * G
                # ---- load per-s data for this super-tile (partition-last on host
                # means partition-major in SBUF: [128, G, inner]) ----
                nbr_sb = io_p.tile([128, G, DEG], mybir.dt.int32, tag="nbr")
                w_sb = io_p.tile([128, G, DEG], f32, tag="w")
                r_sb = io_p.tile([128, G, 9], f32, tag="r")
                tt_sb = io_p.tile([128, G, 3], f32, tag="t")
                v_sb = io_p.tile([128, G, 3], f32, tag="v")
                # DRAM [128, G, inner] -> SBUF [128, G, inner] (contiguous slabs)
                nc.sync.dma_start(out=nbr_sb[:, :, :], in_=nbr_d[:, t0:t0 + G, :])
                nc.sync.dma_start(out=w_sb[:, :, :], in_=w_d[:, t0:t0 + G, :])
                nc.sync.dma_start(out=r_sb[:, :, :], in_=r_d[:, t0:t0 + G, :])
                nc.sync.dma_start(out=tt_sb[:, :, :], in_=t_d[:, t0:t0 + G, :])
                nc.sync.dma_start(out=v_sb[:, :, :], in_=v_d[:, t0:t0 + G, :])

                # ---- gather vn[128, G, DEG, 3] one [128]-row batch at a time ----
                vn = gat_p.tile([128, G, DEG, 3], f32, tag="vn")
                for g in range(G):
                    for d in range(DEG):
                        nc.gpsimd.indirect_dma_start(
                            out=vn[:, g, d, :],
                            out_offset=None,
                            in_=verts_d[:, :],
                            in_offset=bass.IndirectOffsetOnAxis(
                                ap=nbr_sb[:, g, d:d + 1], axis=0),
                        )

                # ---- wv = vn * w (broadcast w over xyz via 3 strided ops) ----
                for c in range(3):
                    nc.vector.tensor_tensor(
                        out=vn[:, :, :, c], in0=vn[:, :, :, c], in1=w_sb[:, :, :],
                        op=mybir.AluOpType.mult)
                # ---- u = sum_d wv ; W = sum_d w  (halving trees over DEG) ----
                h = DEG
                while h > 1:
                    h //= 2
                    nc.vector.tensor_tensor(
                        out=vn[:, :, 0:h, :], in0=vn[:, :, 0:h, :],
                        in1=vn[:, :, h:2 * h, :], op=mybir.AluOpType.add)
                    nc.vector.tensor_tensor(
                        out=w_sb[:, :, 0:h], in0=w_sb[:, :, 0:h],
                        in1=w_sb[:, :, h:2 * h], op=mybir.AluOpType.add)
                # u = vn[:, :, 0, :] ([128, G, 3]), W = w_sb[:, :, 0] ([128, G])

                # ---- dense math ----
                # rv_i = sum_j R[:, :, 3i+j] * v_j ; ru_i likewise with u
                acc = scr_p.tile([128, G, 3], f32, tag="acc")     # R@v
                accu = scr_p.tile([128, G, 3], f32, tag="accu")   # R@u
                prod = scr_p.tile([128, G], f32, tag="prod")
                for i in range(3):
                    for j in range(3):
                        nc.vector.tensor_tensor(
                            out=(acc[:, :, i] if j == 0 else prod[:, :]),
                            in0=r_sb[:, :, 3 * i + j], in1=v_sb[:, :, j],
                            op=mybir.AluOpType.mult)
                        if j > 0:
                            nc.vector.tensor_tensor(
                                out=acc[:, :, i], in0=acc[:, :, i], in1=prod[:, :],
                                op=mybir.AluOpType.add)
                    for j in range(3):
                        nc.vector.tensor_tensor(
                            out=(accu[:, :, i] if j == 0 else prod[:, :]),
                            in0=r_sb[:, :, 3 * i + j], in1=vn[:, :, 0, j],
                            op=mybir.AluOpType.mult)
                        if j > 0:
                            nc.vector.tensor_tensor(
                                out=accu[:, :, i], in0=accu[:, :, i], in1=prod[:, :],
                                op=mybir.AluOpType.add)
                # acc = (R@v + t) * W  - accu + u
                nc.vector.tensor_tensor(out=acc[:, :, :], in0=acc[:, :, :],
                                        in1=tt_sb[:, :, :], op=mybir.AluOpType.add)
                for c in range(3):
                    nc.vector.tensor_tensor(out=acc[:, :, c], in0=acc[:, :, c],
                                            in1=w_sb[:, :, 0], op=mybir.AluOpType.mult)
                nc.vector.tensor_tensor(out=acc[:, :, :], in0=acc[:, :, :],
                                        in1=accu[:, :, :], op=mybir.AluOpType.subtract)
                nc.vector.tensor_tensor(out=acc[:, :, :], in0=acc[:, :, :],
                                        in1=vn[:, :, 0, :], op=mybir.AluOpType.add)
                # ---- store: SBUF [128, G, 3] -> DRAM [128, G, 3] ----
                nc.sync.dma_start(out=out_d[:, t0:t0 + G, :], in_=acc[:, :, :])
    nc.compile()
    return nc


def kernel(verts, R, t, w, sampled_ids, neighbor_ids):
    from concourse.bass_utils import run_bass_kernel_spmd

    verts = np.ascontiguousarray(np.asarray(verts, dtype=np.float32))
    R = np.asarray(R, dtype=np.float32)
    t = np.asarray(t, dtype=np.float32)
    w = np.asarray(w, dtype=np.float32)
    ids_dtype = np.asarray(sampled_ids).dtype
    sampled = np.asarray(sampled_ids).astype(np.int64)
    nbr = np.asarray(neighbor_ids).astype(np.int32)

    if "nc" not in _nc_cache:
        _nc_cache["nc"] = _build_nc()
    nc = _nc_cache["nc"]

    v = verts[sampled]  # [S, 3] sampled vertex positions (arange ids -> slice)

    def shard(arr, inner):
        """arr [S, inner...] -> per-core [TILES, inner, 128] partition-last."""
        out = []
        for c in range(N_CORES):
            a = arr[c * S_SH:(c + 1) * S_SH].reshape(S_SH, inner)
            pad = np.zeros((S_PAD - S_SH, inner), a.dtype)
            a = np.concatenate([a, pad], axis=0) if S_PAD != S_SH else a
            out.append(np.ascontiguousarray(
                a.reshape(TILES, 128, inner).transpose(1, 0, 2)))
        return out

    nbr_sh = shard(np.clip(nbr, 0, NV - 1), DEG)
    w_sh = shard(w, DEG)
    r_sh = shard(R.reshape(-1, 9), 9)
    t_sh = shard(t, 3)
    v_sh = shard(v, 3)

    in_maps = [{"verts": verts, "nbr": nbr_sh[c], "w": w_sh[c], "r": r_sh[c],
                "t": t_sh[c], "v": v_sh[c]} for c in range(N_CORES)]
    res = run_bass_kernel_spmd(nc, in_maps, core_ids=list(range(N_CORES)))

    new_verts = np.empty((S_FULL, 3), np.float32)
    for c in range(N_CORES):
        o = res.results[c]["out"]  # [128, TILES, 3]
        o = o.transpose(1, 0, 2).reshape(S_PAD, 3)[:S_SH]
        new_verts[c * S_SH:(c + 1) * S_SH] = o

    updated = verts.copy()
    updated[sampled] = new_verts
    return new_verts, updated


# revision 3
# speedup vs baseline: 823.5090x; 823.5090x over previous
"""Trainium2 Bass kernel for nn_DeformParams (gnn_message_passing).

Math (per sampled vertex s, neighbors d=0..15):
    rotated[s,d] = R[s] @ (v[s] - vn[s,d])
    new_verts[s] = sum_d w[s,d] * (rotated[s,d] + vn[s,d] + t[s])
which factors to
    u[s]  = sum_d w[s,d] * vn[s,d]          (the only gather-dependent term)
    W[s]  = sum_d w[s,d]
    new_verts[s] = W*(R@v + t) - R@u + u
    updated = verts with rows sampled_ids replaced by new_verts

Distribution: sampled vertices (S=500k) sharded across 8 NeuronCores; verts
table replicated (stays in HBM, rows fetched by indirect DMA gather).

Device kernel per core: for each tile of 128 sampled vertices x 16 neighbors,
gather the 128 neighbor rows per d via gpsimd indirect DMA ([128,1]-offset
form, one row per partition), then DVE does the weighted reduction and the
3x3 dense math. Output is the core's new_verts slice; `updated` is assembled
on the host from new_verts + untouched verts rows (output assembly only).
"""
import numpy as np

N_CORES = 8
S_FULL = 500_000
DEG = 16
NV = 2_000_000

S_SH = S_FULL // N_CORES          # 62500 sampled verts per core
TILES = (S_SH + 127) // 128       # 489 tiles of 128
S_PAD = TILES * 128               # 62592 (padded)
G = 163                           # s-tiles per super-tile; 489 = 3 * 163
N_SUPER = TILES // G              # 3

_nc_cache = {}


def _build_nc():
    import concourse.bacc as bacc
    import concourse.bass as bass
    import concourse.mybir as mybir
    import concourse.tile as tile

    nc = bacc.Bacc("TRN2", target_bir_lowering=False, debug=False,
                   num_devices=N_CORES)
    f32 = mybir.dt.float32
    verts_d = nc.dram_tensor("verts", [NV, 3], f32, kind="ExternalInput")
    # per-core, s-padded, partition-last layouts (host pre-transposed)
    nbr_d = nc.dram_tensor("nbr", [128, TILES, DEG], mybir.dt.int32, kind="ExternalInput")
    w_d = nc.dram_tensor("w", [128, TILES, DEG], f32, kind="ExternalInput")
    r_d = nc.dram_tensor("r", [128, TILES, 9], f32, kind="ExternalInput")
    t_d = nc.dram_tensor("t", [128, TILES, 3], f32, kind="ExternalInput")
    v_d = nc.dram_tensor("v", [128, TILES, 3], f32, kind="ExternalInput")
    out_d = nc.dram_tensor("out", [128, TILES, 3], f32, kind="ExternalOutput")

    with tile.TileContext(nc) as tc:
        with tc.tile_pool(name="io", bufs=2) as io_p, \
             tc.tile_pool(name="gat", bufs=2) as gat_p, \
             tc.tile_pool(name="scr", bufs=2) as scr_p:
            for sup in range(N_SUPER):
                t0 = sup * G
                # ---- load per-s data for this super-tile (partition-last on host
                # means partition-major in SBUF: [128, G, inner]) ----
                nbr_sb = io_p.tile([128, G, DEG], mybir.dt.int32, tag="nbr")
                w_sb = io_p.tile([128, G, DEG], f32, tag="w")
                r_sb = io_p.tile([128, G, 9], f32, tag="r")
                tt_sb = io_p.tile([128, G, 3], f32, tag="t")
                v_sb = io_p.tile([128, G, 3], f32, tag="v")
                # DRAM [128, G, inner] -> SBUF [128, G, inner] (contiguous slabs)
                nc.sync.dma_start(out=nbr_sb[:, :, :], in_=nbr_d[:, t0:t0 + G, :])
                nc.sync.dma_start(out=w_sb[:, :, :], in_=w_d[:, t0:t0 + G, :])
                nc.sync.dma_start(out=r_sb[:, :, :], in_=r_d[:, t0:t0 + G, :])
                nc.sync.dma_start(out=tt_sb[:, :, :], in_=t_d[:, t0:t0 + G, :])
                nc.sync.dma_start(out=v_sb[:, :, :], in_=v_d[:, t0:t0 + G, :])

                # ---- gather vn[128, G, DEG, 3] one [128]-row batch at a time ----
                vn = gat_p.tile([128, G, DEG, 3], f32, tag="vn")
                for g in range(G):
                    for d in range(DEG):
                        nc.gpsimd.indirect_dma_start(
                            out=vn[:, g, d, :],
                            out_offset=None,
                            in_=verts_d[:, :],
                            in_offset=bass.IndirectOffsetOnAxis(
                                ap=nbr_sb[:, g, d:d + 1], axis=0),
                        )

                # ---- wv = vn * w (broadcast w over xyz via 3 strided ops) ----
                for c in range(3):
                    nc.vector.tensor_tensor(
                        out=vn[:, :, :, c], in0=vn[:, :, :, c], in1=w_sb[:, :, :],
                        op=mybir.AluOpType.mult)
                # ---- u = sum_d wv ; W = sum_d w  (halving trees over DEG) ----
                h = DEG
                while h > 1:
                    h //= 2
                    nc.vector.tensor_tensor(
                        out=vn[:, :, 0:h, :], in0=vn[:, :, 0:h, :],
                        in1=vn[:, :, h:2 * h, :], op=mybir.AluOpType.add)
                    nc.vector.tensor_tensor(
                        out=w_sb[:, :, 0:h], in0=w_sb[:, :, 0:h],
                        in1=w_sb[:, :, h:2 * h], op=mybir.AluOpType.add)
                # u = vn[:, :, 0, :] ([128, G, 3]), W = w_sb[:, :, 0] ([128, G])

                # ---- dense math ----
                # rv_i = sum_j R[:, :, 3i+j] * v_j ; ru_i likewise with u
                acc = scr_p.tile([128, G, 3], f32, tag="acc")     # R@v
                accu = scr_p.tile([128, G, 3], f32, tag="accu")   # R@u
                prod = scr_p.tile([128, G], f32, tag="prod")
                for i in range(3):
                    for j in range(3):
                        nc.vector.tensor_tensor(
                            out=(acc[:, :, i] if j == 0 else prod[:, :]),
                            in0=r_sb[:, :, 3 * i + j], in1=v_sb[:, :, j],
                            op=mybir.AluOpType.mult)
                        if j > 0:
                            nc.vector.tensor_tensor(
                                out=acc[:, :, i], in0=acc[:, :, i], in1=prod[:, :],
                                op=mybir.AluOpType.add)
                    for j in range(3):
                        nc.vector.tensor_tensor(
                            out=(accu[:, :, i] if j == 0 else prod[:, :]),
                            in0=r_sb[:, :, 3 * i + j], in1=vn[:, :, 0, j],
                            op=mybir.AluOpType.mult)
                        if j > 0:
                            nc.vector.tensor_tensor(
                                out=accu[:, :, i], in0=accu[:, :, i], in1=prod[:, :],
                                op=mybir.AluOpType.add)
                # acc = (R@v + t) * W  - accu + u
                nc.vector.tensor_tensor(out=acc[:, :, :], in0=acc[:, :, :],
                                        in1=tt_sb[:, :, :], op=mybir.AluOpType.add)
                for c in range(3):
                    nc.vector.tensor_tensor(out=acc[:, :, c], in0=acc[:, :, c],
                                            in1=w_sb[:, :, 0], op=mybir.AluOpType.mult)
                nc.vector.tensor_tensor(out=acc[:, :, :], in0=acc[:, :, :],
                                        in1=accu[:, :, :], op=mybir.AluOpType.subtract)
                nc.vector.tensor_tensor(out=acc[:, :, :], in0=acc[:, :, :],
                                        in1=vn[:, :, 0, :], op=mybir.AluOpType.add)
                # ---- store: SBUF [128, G, 3] -> DRAM [128, G, 3] ----
                nc.sync.dma_start(out=out_d[:, t0:t0 + G, :], in_=acc[:, :, :])
    nc.compile()
    return nc


def kernel(verts, R, t, w, sampled_ids, neighbor_ids):
    from concourse.bass_utils import run_bass_kernel_spmd

    verts = np.ascontiguousarray(np.asarray(verts, dtype=np.float32))
    R = np.asarray(R, dtype=np.float32)
    t = np.asarray(t, dtype=np.float32)
    w = np.asarray(w, dtype=np.float32)
    sampled = np.asarray(sampled_ids).astype(np.int64)
    nbr = np.asarray(neighbor_ids).astype(np.int32)

    if "nc" not in _nc_cache:
        _nc_cache["nc"] = _build_nc()
    nc = _nc_cache["nc"]

    v = verts[sampled]  # [S, 3] sampled vertex positions (arange ids -> slice)

    def shard(arr, inner):
        """arr [S, inner...] -> per-core [TILES, inner, 128] partition-last."""
        out = []
        for c in range(N_CORES):
            a = arr[c * S_SH:(c + 1) * S_SH].reshape(S_SH, inner)
            pad = np.zeros((S_PAD - S_SH, inner), a.dtype)
            a = np.concatenate([a, pad], axis=0) if S_PAD != S_SH else a
            out.append(np.ascontiguousarray(
                a.reshape(TILES, 128, inner).transpose(1, 0, 2)))
        return out

    nbr_sh = shard(np.clip(nbr, 0, NV - 1), DEG)
    w_sh = shard(w, DEG)
    r_sh = shard(R.reshape(-1, 9), 9)
    t_sh = shard(t, 3)
    v_sh = shard(v, 3)

    in_maps = [{"verts": verts, "nbr": nbr_sh[c], "w": w_sh[c], "r": r_sh[c],
                "t": t_sh[c], "v": v_sh[c]} for c in range(N_CORES)]
    globals()["_last_in_maps"] = in_maps
    res = run_bass_kernel_spmd(nc, in_maps, core_ids=list(range(N_CORES)))

    new_verts = np.empty((S_FULL, 3), np.float32)
    for c in range(N_CORES):
        o = res.results[c]["out"]  # [128, TILES, 3]
        o = o.transpose(1, 0, 2).reshape(S_PAD, 3)[:S_SH]
        new_verts[c * S_SH:(c + 1) * S_SH] = o

    updated = verts.copy()
    updated[sampled] = new_verts
    return new_verts, updated
